# revision 5
# baseline (speedup 1.0000x reference)
"""Trainium2 Bass kernel for nn_CropRoi (FPN ROI crop / roi-align style).

Contract: kernel(**inputs) takes the FULL inputs (p2..p5 feature pyramid,
proposals [1024, 7]) and returns the FULL output [1024, 256, 14, 14] f32.

Strategy
--------
Host (numpy, proposal-derived only — cheap):
  * Transpose each pyramid level to channels-last and concatenate into one
    flat pixel table featcat [174080, 256] f32 (one pixel = 1KB contiguous).
  * Route each proposal to its pyramid level, compute its source patch
    bounding box and separable bilinear weights wy[row, i], wx[col, j].
  * Flatten each patch to a pixel list; pack pixels into 128-pixel "slots"
    (1-3 slots per proposal), pad with zero-weight entries.
  * Distribute 128 proposals to each of the 8 cores (static slot schedule).

Device (one static SPMD Bass/Tile program on 8 cores):
  Per slot: indirect-DMA gather of 128 pixels (1KB descriptors) into an
  SBUF patch [128 pix, 256 ch]; build the dense 2D interp weight matrix
  W2D[pix, (i,j)] = wy[pix,i]*wx[pix,j] with one DVE broadcast-multiply;
  two fp32 matmuls patch.T @ W2D accumulate the crop [256 ch, 196] in PSUM.
  Per proposal: copy PSUM -> SBUF and DMA the contiguous 200KB crop out.
"""

import os
import sys

for _p in ("/opt/trn_rl_repo",):
    if os.path.isdir(_p) and _p not in sys.path:
        sys.path.insert(0, _p)

import numpy as np

import concourse.bass as bass
import concourse.bacc as bacc
import concourse.mybir as mybir
from concourse.tile import TileContext
from concourse import bass_utils

# ---------------------------------------------------------------- constants
IMG = 1024
CS = 14  # crop size
STRIDES = (4, 8, 16, 32)
BASE_SIZES = (8.0, 16.0, 32.0, 64.0)
B = 2
C = 256
NPROP = 1024
NCORES = 8
PPC = NPROP // NCORES  # proposals per core = 128

HWL = [IMG // s for s in STRIDES]  # [256, 128, 64, 32]
NPIXL = [B * h * h for h in HWL]
LEVEL_OFF = np.cumsum([0] + NPIXL)[:4]
TOTALPIX = int(sum(NPIXL))  # 174080

SLOT_PIX = 128
# per-core static slot schedule: group g owns SCHED[g] consecutive slots
N3, N2 = 3, 16
SCHED = [1] * (PPC - N2 - N3) + [2] * N2 + [3] * N3
SLOT_START = np.cumsum([0] + SCHED)[:-1]
NSLOT = int(sum(SCHED))  # 150

_F32 = mybir.dt.float32
_I32 = mybir.dt.int32


# ---------------------------------------------------------------- planner
def _plan_proposals(proposals):
    """Per-proposal gather indices + separable bilinear weight factors."""
    pr = np.asarray(proposals, dtype=np.float32)
    n = pr.shape[0]
    bi = pr[:, 0].astype(np.int32)
    x0, y0, x1, y1 = pr[:, 1], pr[:, 2], pr[:, 3], pr[:, 4]
    sizes = np.sqrt((x1 - x0) * (y1 - y0))
    base = np.asarray(BASE_SIZES, np.float32)
    lvl = np.argmin(np.abs(sizes[:, None] - base[None, :]), axis=1)
    grid = (np.arange(CS, dtype=np.float32) / np.float32(CS - 1))
    ar = np.arange(CS)

    plans = []
    for i in range(n):
        l = int(lvl[i])
        H = HWL[l]
        s = np.float32(1.0 / STRIDES[l])
        ys = y0[i] * s + (y1[i] - y0[i]) * s * grid
        xs = x0[i] * s + (x1[i] - x0[i]) * s * grid
        yf = np.floor(ys)
        xf = np.floor(xs)
        ly = ys - yf
        lx = xs - xf
        yi0 = np.clip(yf.astype(np.int64), 0, H - 1)
        yi1 = np.clip(yi0 + 1, 0, H - 1)
        xi0 = np.clip(xf.astype(np.int64), 0, H - 1)
        xi1 = np.clip(xi0 + 1, 0, H - 1)
        ylo = int(yi0.min())
        hp = int(yi1.max()) - ylo + 1
        xlo = int(xi0.min())
        wp = int(xi1.max()) - xlo + 1
        wyrow = np.zeros((hp, CS), np.float32)
        np.add.at(wyrow, (yi0 - ylo, ar), 1.0 - ly)
        np.add.at(wyrow, (yi1 - ylo, ar), ly)
        wxcol = np.zeros((wp, CS), np.float32)
        np.add.at(wxcol, (xi0 - xlo, ar), 1.0 - lx)
        np.add.at(wxcol, (xi1 - xlo, ar), lx)
        pbase = int(LEVEL_OFF[l]) + int(bi[i]) * H * H
        idx = (pbase + (ylo + np.arange(hp))[:, None] * H
               + (xlo + np.arange(wp))[None, :]).reshape(-1).astype(np.int32)
        wy2 = np.repeat(wyrow, wp, axis=0)  # [hp*wp, 14]
        wx2 = np.tile(wxcol, (hp, 1))       # [hp*wp, 14]
        plans.append((idx, wy2, wx2))
    return plans, lvl


def _assign_cores(plans):
    """Place each proposal into a (core, group) with enough slots.

    Returns placement[core][group] = prop id (or None) and the list of
    proposals that did not fit (host fallback — normally empty).
    """
    nchunks = [min((len(p[0]) + SLOT_PIX - 1) // SLOT_PIX, 10) for p in plans]
    fallback = [i for i, c in enumerate(nchunks) if c > 3]
    placement = [[None] * PPC for _ in range(NCORES)]
    free = []
    for _ in range(NCORES):
        free.append({
            1: list(range(0, PPC - N2 - N3)),
            2: list(range(PPC - N2 - N3, PPC - N3)),
            3: list(range(PPC - N3, PPC)),
        })

    def place(pid, need):
        # try round-robin over cores, groups with >= need slots
        for attempt in range(NCORES):
            core = place.cursor % NCORES
            place.cursor += 1
            f = free[core]
            for cap in range(need, 4):
                if f[cap]:
                    g = f[cap].pop(0)
                    placement[core][g] = pid
                    return True
        return False

    place.cursor = 0
    for need in (3, 2, 1):
        for pid in range(len(plans)):
            if nchunks[pid] == need and pid not in fallback:
                if not place(pid, need):
                    fallback.append(pid)
    return placement, fallback


def _build_slot_tensors(plans, placement):
    idxT = np.zeros((NCORES, 128, NSLOT), np.int32)
    wyx = np.zeros((NCORES, 128, NSLOT * 28), np.float32)
    for core in range(NCORES):
        for g in range(PPC):
            pid = placement[core][g]
            if pid is None:
                continue
            idx, wy2, wx2 = plans[pid]
            npx = len(idx)
            for k in range(SCHED[g]):
                lo = k * SLOT_PIX
                if lo >= npx:
                    break
                hi = min(npx, lo + SLOT_PIX)
                cnt = hi - lo
                s = int(SLOT_START[g]) + k
                idxT[core, :cnt, s] = idx[lo:hi]
                wyx[core, :cnt, s * 28:s * 28 + 14] = wy2[lo:hi]
                wyx[core, :cnt, s * 28 + 14:s * 28 + 28] = wx2[lo:hi]
    return idxT, wyx


def _host_crop(feats_cl, plans, pid):
    """Numpy fallback for proposals that exceed slot capacity (rare)."""
    idx, wy2, wx2 = plans[pid]
    pix = feats_cl[idx]  # [npx, 256]
    w2d = wy2[:, :, None] * wx2[:, None, :]  # [npx, 14, 14]
    out = np.einsum("pc,pij->cij", pix, w2d.reshape(len(idx), CS * CS)
                    .reshape(len(idx), CS, CS))
    return out.astype(np.float32)


# ---------------------------------------------------------------- device
def build_bass_program(totalpix=TOTALPIX, sched=None, nprops=PPC):
    sched = list(SCHED) if sched is None else list(sched)
    nslot = int(sum(sched))
    nc = bacc.Bacc("TRN2", target_bir_lowering=False)
    feat = nc.dram_tensor("featcat", [totalpix, C], _F32, kind="ExternalInput")
    idx = nc.dram_tensor("idx", [128, nslot], _I32, kind="ExternalInput")
    wyx = nc.dram_tensor("wyx", [128, nslot * 28], _F32, kind="ExternalInput")
    out = nc.dram_tensor("out", [nprops, C, CS * CS], _F32,
                         kind="ExternalOutput")

    with TileContext(nc) as tc:
        with tc.tile_pool(name="const", bufs=1) as cpool, \
             tc.tile_pool(name="patch", bufs=6) as ppool, \
             tc.tile_pool(name="w2d", bufs=6) as wpool, \
             tc.tile_pool(name="stage", bufs=4) as spool, \
             tc.tile_pool(name="psum", bufs=4, space="PSUM") as qpool:
            idx_sb = cpool.tile([128, nslot], _I32)
            nc.sync.dma_start(out=idx_sb[:], in_=idx[:, :])
            wyx_sb = cpool.tile([128, nslot * 28], _F32)
            nc.sync.dma_start(out=wyx_sb[:], in_=wyx[:, :])

            s = 0
            for g in range(nprops):
                k = sched[g]
                psA = qpool.tile([128, CS * CS], _F32, tag="psA")
                psB = qpool.tile([128, CS * CS], _F32, tag="psB")
                for ci in range(k):
                    patch = ppool.tile([128, C], _F32)
                    nc.gpsimd.indirect_dma_start(
                        out=patch[:],
                        out_offset=None,
                        in_=feat[:, :],
                        in_offset=bass.IndirectOffsetOnAxis(
                            ap=idx_sb[:, s:s + 1], axis=0),
                    )
                    w2d = wpool.tile([128, CS * CS], _F32)
                    wyb = wyx_sb[:, s * 28:s * 28 + 14] \
                        .unsqueeze(2).to_broadcast([128, CS, CS])
                    wxb = wyx_sb[:, s * 28 + 14:s * 28 + 28] \
                        .unsqueeze(1).to_broadcast([128, CS, CS])
                    nc.vector.tensor_tensor(
                        out=w2d[:].rearrange("p (i j) -> p i j", i=CS),
                        in0=wyb, in1=wxb, op=mybir.AluOpType.mult)
                    nc.tensor.matmul(psA[:],
                                     lhsT=patch[:, 0:128], rhs=w2d[:],
                                     start=(ci == 0), stop=(ci == k - 1))
                    nc.tensor.matmul(psB[:],
                                     lhsT=patch[:, 128:256], rhs=w2d[:],
                                     start=(ci == 0), stop=(ci == k - 1))
                    s += 1
                stage = spool.tile([128, 2 * CS * CS], _F32)
                nc.vector.tensor_copy(out=stage[:, 0:CS * CS], in_=psA[:])
                nc.vector.tensor_copy(out=stage[:, CS * CS:2 * CS * CS],
                                      in_=psB[:])
                nc.sync.dma_start(
                    out=out[g].rearrange("(h p) t -> p h t", p=128),
                    in_=stage[:].rearrange("p (h t) -> p h t", h=2))
    nc.finalize()
    return nc


_NC_CACHE = {}


def _get_program():
    key = "main"
    if key not in _NC_CACHE:
        _NC_CACHE[key] = build_bass_program()
    return _NC_CACHE[key]


# ---------------------------------------------------------------- entry
def _prepare(p2, p3, p4, p5, proposals):
    feats = [np.asarray(p, np.float32) for p in (p2, p3, p4, p5)]
    featcat = np.concatenate(
        [np.ascontiguousarray(f.transpose(0, 2, 3, 1)).reshape(-1, C)
         for f in feats], axis=0)
    plans, _lvl = _plan_proposals(proposals)
    placement, fallback = _assign_cores(plans)
    idxT, wyx = _build_slot_tensors(plans, placement)
    return featcat, plans, placement, fallback, idxT, wyx


def run(p2, p3, p4, p5, proposals, trace=False):
    featcat, plans, placement, fallback, idxT, wyx = _prepare(
        p2, p3, p4, p5, proposals)
    nc = _get_program()
    in_maps = [
        {"featcat": featcat, "idx": idxT[c], "wyx": wyx[c]}
        for c in range(NCORES)
    ]
    res = bass_utils.run_bass_kernel_spmd(
        nc, in_maps, core_ids=list(range(NCORES)), trace=trace)
    out = np.empty((NPROP, C, CS, CS), np.float32)
    for core in range(NCORES):
        core_out = res.results[core]["out"]  # [PPC, 256, 196]
        for g in range(PPC):
            pid = placement[core][g]
            if pid is None:
                continue
            out[pid] = core_out[g].reshape(C, CS, CS)
    for pid in fallback:
        out[pid] = _host_crop(featcat, plans, pid)
    return out, res


def kernel(p2, p3, p4, p5, proposals):
    out, _res = run(p2, p3, p4, p5, proposals, trace=False)
    return out


# revision 7
# speedup vs baseline: 1.4610x; 1.4610x over previous
"""Trainium2 Bass kernel for nn_CropRoi (FPN ROI crop / roi-align style).

Contract: kernel(**inputs) takes the FULL inputs (p2..p5 feature pyramid,
proposals [1024, 7]) and returns the FULL output [1024, 256, 14, 14] f32.

Strategy
--------
Host (numpy, proposal-derived only — cheap):
  * Transpose each pyramid level to channels-last and concatenate into one
    flat pixel table featcat [174080, 256] f32 (one pixel = 1KB contiguous).
  * Route each proposal to its level, compute its source patch bounding box
    and separable bilinear weights wy[row, i], wx[col, j].
  * Flatten each patch to a pixel list (hp*wp pixels, mean ~45) and pack
    into 128-pixel slots. Small proposals are PAIRED two-per-slot (block
    structure handled by host-zeroed weight factors); big ones span 2-3
    slots, accumulated in PSUM.
  * Distribute proposals over 8 cores into a STATIC per-core group
    schedule: TP pair-groups (1 slot, 2 props), T1 solo (1 slot),
    T2 (2 slots), T3 (3 slots).

Device (one static SPMD Bass/Tile program on 8 cores):
  Per group: one indirect-DMA gather of its 128*k pixels (1KB descriptors,
  channels-last) into SBUF [128 pix, k*256 ch]; per slot build the 2D
  interp weight matrix W2D[pix, (g,i,j)] = wy[pix,(g,i)]*wx[pix,(g,j)]
  with one DVE broadcast-multiply; two fp32 matmuls patch.T @ W2D
  accumulate the crop(s) [256 ch, g*196] in PSUM; copy PSUM -> SBUF
  (DVE + ACT split) and DMA contiguous crops out.
"""

import os
import sys

for _p in ("/opt/trn_rl_repo",):
    if os.path.isdir(_p) and _p not in sys.path:
        sys.path.insert(0, _p)

import numpy as np

import concourse.bass as bass
import concourse.bacc as bacc
import concourse.mybir as mybir
from concourse.tile import TileContext
from concourse import bass_utils

# ---------------------------------------------------------------- constants
IMG = 1024
CS = 14  # crop size
TT = CS * CS  # 196
STRIDES = (4, 8, 16, 32)
BASE_SIZES = (8.0, 16.0, 32.0, 64.0)
B = 2
C = 256
NPROP = 1024
NCORES = 8
PPC = NPROP // NCORES  # proposals per core = 128

HWL = [IMG // s for s in STRIDES]  # [256, 128, 64, 32]
NPIXL = [B * h * h for h in HWL]
LEVEL_OFF = np.cumsum([0] + NPIXL)[:4]
TOTALPIX = int(sum(NPIXL))  # 174080

SLOT_PIX = 128

# static per-core group schedule: (kind, nslots, nrows)
N_TP, N_T1, N_T2, N_T3 = 59, 8, 11, 2
GROUPS = ([("TP", 1, 2)] * N_TP + [("T1", 1, 1)] * N_T1
          + [("T2", 2, 1)] * N_T2 + [("T3", 3, 1)] * N_T3)
NG = len(GROUPS)
NSLOT = sum(g[1] for g in GROUPS)       # 95
NROWS = sum(g[2] for g in GROUPS)       # 139
GROUP_SLOT = np.cumsum([0] + [g[1] for g in GROUPS])[:-1]
GROUP_ROW = np.cumsum([0] + [g[2] for g in GROUPS])[:-1]

_F32 = mybir.dt.float32
_I32 = mybir.dt.int32


# ---------------------------------------------------------------- planner
def _plan_proposals(proposals):
    """Per-proposal gather indices + separable bilinear weight factors."""
    pr = np.asarray(proposals, dtype=np.float32)
    n = pr.shape[0]
    bi = pr[:, 0].astype(np.int32)
    x0, y0, x1, y1 = pr[:, 1], pr[:, 2], pr[:, 3], pr[:, 4]
    sizes = np.sqrt((x1 - x0) * (y1 - y0))
    base = np.asarray(BASE_SIZES, np.float32)
    lvl = np.argmin(np.abs(sizes[:, None] - base[None, :]), axis=1)
    grid = (np.arange(CS, dtype=np.float32) / np.float32(CS - 1))
    ar = np.arange(CS)

    plans = []
    for i in range(n):
        l = int(lvl[i])
        H = HWL[l]
        s = np.float32(1.0 / STRIDES[l])
        ys = y0[i] * s + (y1[i] - y0[i]) * s * grid
        xs = x0[i] * s + (x1[i] - x0[i]) * s * grid
        yf = np.floor(ys)
        xf = np.floor(xs)
        ly = ys - yf
        lx = xs - xf
        yi0 = np.clip(yf.astype(np.int64), 0, H - 1)
        yi1 = np.clip(yi0 + 1, 0, H - 1)
        xi0 = np.clip(xf.astype(np.int64), 0, H - 1)
        xi1 = np.clip(xi0 + 1, 0, H - 1)
        ylo = int(yi0.min())
        hp = int(yi1.max()) - ylo + 1
        xlo = int(xi0.min())
        wp = int(xi1.max()) - xlo + 1
        wyrow = np.zeros((hp, CS), np.float32)
        np.add.at(wyrow, (yi0 - ylo, ar), 1.0 - ly)
        np.add.at(wyrow, (yi1 - ylo, ar), ly)
        wxcol = np.zeros((wp, CS), np.float32)
        np.add.at(wxcol, (xi0 - xlo, ar), 1.0 - lx)
        np.add.at(wxcol, (xi1 - xlo, ar), lx)
        pbase = int(LEVEL_OFF[l]) + int(bi[i]) * H * H
        idx = (pbase + (ylo + np.arange(hp))[:, None] * H
               + (xlo + np.arange(wp))[None, :]).reshape(-1).astype(np.int32)
        wy2 = np.repeat(wyrow, wp, axis=0)  # [hp*wp, 14]
        wx2 = np.tile(wxcol, (hp, 1))       # [hp*wp, 14]
        plans.append((idx, wy2, wx2))
    return plans, lvl


def _assign_cores(plans):
    """Assign proposals to (core, group, sub) positions.

    Returns placement[core][g] = list of prop ids occupying that group
    (len 1 or 2 for TP), and the host-fallback list.
    """
    n = len(plans)
    pix = np.array([len(p[0]) for p in plans])
    ch = np.minimum((pix + SLOT_PIX - 1) // SLOT_PIX, 9)
    fallback = [i for i in range(n) if ch[i] > 3]

    core_ids = [[] for _ in range(NCORES)]
    for need in (3, 2, 1):
        ids = [i for i in range(n) if ch[i] == need and i not in fallback]
        # deal round-robin, smallest cores first for balance
        order = np.argsort([len(c) for c in core_ids], kind="stable")
        j = 0
        for i in ids:
            core_ids[order[j % NCORES]].append(i)
            j += 1

    placement = [[None] * NG for _ in range(NCORES)]
    kinds = [g[0] for g in GROUPS]
    tp_g = [i for i, k in enumerate(kinds) if k == "TP"]
    t1_g = [i for i, k in enumerate(kinds) if k == "T1"]
    t2_g = [i for i, k in enumerate(kinds) if k == "T2"]
    t3_g = [i for i, k in enumerate(kinds) if k == "T3"]

    for core in range(NCORES):
        ids = core_ids[core]
        free_tp = list(tp_g)
        free_t1 = list(t1_g)
        free_t2 = list(t2_g)
        free_t3 = list(t3_g)
        c3 = [i for i in ids if ch[i] == 3]
        c2 = [i for i in ids if ch[i] == 2]
        c1 = sorted((i for i in ids if ch[i] == 1), key=lambda i: pix[i])
        for i in c3:
            if free_t3:
                placement[core][free_t3.pop(0)] = [i]
            else:
                fallback.append(i)
        for i in c2:
            if free_t2:
                placement[core][free_t2.pop(0)] = [i]
            elif free_t3:
                placement[core][free_t3.pop(0)] = [i]
            else:
                fallback.append(i)
        # two-pointer pairing of the 1-chunk proposals
        lo, hi = 0, len(c1) - 1
        pairs, solos = [], []
        while lo < hi:
            if pix[c1[lo]] + pix[c1[hi]] <= SLOT_PIX:
                pairs.append((c1[lo], c1[hi]))
                lo += 1
                hi -= 1
            else:
                solos.append(c1[hi])
                hi -= 1
        if lo == hi:
            solos.append(c1[lo])
        for a, b in pairs:
            if free_tp:
                placement[core][free_tp.pop(0)] = [a, b]
            else:
                solos.extend((a, b))
        for i in solos:
            if free_t1:
                placement[core][free_t1.pop(0)] = [i]
            elif free_tp:
                placement[core][free_tp.pop(0)] = [i]
            elif free_t2:
                placement[core][free_t2.pop(0)] = [i]
            elif free_t3:
                placement[core][free_t3.pop(0)] = [i]
            else:
                fallback.append(i)
    return placement, fallback


def _build_slot_tensors(plans, placement):
    idxT = np.zeros((NCORES, 128, NSLOT), np.int32)
    wyx = np.zeros((NCORES, 128, NSLOT * 56), np.float32)
    for core in range(NCORES):
        for g in range(NG):
            occ = placement[core][g]
            if not occ:
                continue
            kind, nslots, _ = GROUPS[g]
            s0 = int(GROUP_SLOT[g])
            if kind == "TP":
                # sub-proposals share slot s0; rows stacked, weight col
                # block per sub (host-zeroed cross blocks)
                row = 0
                for sub, pid in enumerate(occ):
                    idx, wy2, wx2 = plans[pid]
                    npx = len(idx)
                    idxT[core, row:row + npx, s0] = idx
                    cw = s0 * 56 + sub * 14
                    wyx[core, row:row + npx, cw:cw + 14] = wy2
                    wyx[core, row:row + npx, cw + 28:cw + 42] = wx2
                    row += npx
            else:
                pid = occ[0]
                idx, wy2, wx2 = plans[pid]
                npx = len(idx)
                for q in range(nslots):
                    a = q * SLOT_PIX
                    if a >= npx:
                        break
                    b = min(npx, a + SLOT_PIX)
                    cnt = b - a
                    s = s0 + q
                    idxT[core, :cnt, s] = idx[a:b]
                    wyx[core, :cnt, s * 56:s * 56 + 14] = wy2[a:b]
                    wyx[core, :cnt, s * 56 + 28:s * 56 + 42] = wx2[a:b]
    return idxT, wyx


def _host_crop(featcat, plans, pid):
    """Numpy fallback for proposals that exceed slot capacity (rare)."""
    idx, wy2, wx2 = plans[pid]
    w2d = (wy2[:, :, None] * wx2[:, None, :]).reshape(len(idx), TT)
    out = featcat[idx].T @ w2d
    return out.reshape(C, CS, CS).astype(np.float32)


# ---------------------------------------------------------------- device
def build_bass_program(totalpix=TOTALPIX, groups=None):
    groups = GROUPS if groups is None else groups
    nslot = sum(g[1] for g in groups)
    nrows = sum(g[2] for g in groups)
    g_slot = np.cumsum([0] + [g[1] for g in groups])[:-1]
    g_row = np.cumsum([0] + [g[2] for g in groups])[:-1]

    nc = bacc.Bacc("TRN2", target_bir_lowering=False)
    feat = nc.dram_tensor("featcat", [totalpix, C], _F32, kind="ExternalInput")
    idx = nc.dram_tensor("idx", [128, nslot], _I32, kind="ExternalInput")
    wyx = nc.dram_tensor("wyx", [128, nslot * 56], _F32, kind="ExternalInput")
    out = nc.dram_tensor("out", [nrows, C, TT], _F32, kind="ExternalOutput")

    def w2d_build(w2d_ap, s, ngrp):
        # W2D[p, (g,i,j)] = wy[p, (g,i)] * wx[p, (g,j)]
        wy = wyx_sb[:, s * 56:s * 56 + 14 * ngrp]
        wx = wyx_sb[:, s * 56 + 28:s * 56 + 28 + 14 * ngrp]
        wyb = wy.rearrange("p (g i) -> p g i", g=ngrp) \
            .unsqueeze(3).to_broadcast([128, ngrp, CS, CS])
        wxb = wx.rearrange("p (g j) -> p g j", g=ngrp) \
            .unsqueeze(2).to_broadcast([128, ngrp, CS, CS])
        nc.vector.tensor_tensor(
            out=w2d_ap.rearrange("p (g i j) -> p g i j", g=ngrp, i=CS),
            in0=wyb, in1=wxb, op=mybir.AluOpType.mult)

    with TileContext(nc) as tc:
        with tc.tile_pool(name="const", bufs=1) as cpool, \
             tc.tile_pool(name="patch", bufs=8) as ppool, \
             tc.tile_pool(name="w2d", bufs=8) as wpool, \
             tc.tile_pool(name="stage", bufs=6) as spool, \
             tc.tile_pool(name="psum", bufs=3, space="PSUM") as qpool:
            idx_sb = cpool.tile([128, nslot], _I32)
            nc.sync.dma_start(out=idx_sb[:], in_=idx[:, :])
            wyx_sb = cpool.tile([128, nslot * 56], _F32)
            nc.sync.dma_start(out=wyx_sb[:], in_=wyx[:, :])

            for g, (kind, nslots, nrow) in enumerate(groups):
                s0 = int(g_slot[g])
                r0 = int(g_row[g])
                psA = qpool.tile([128, 392], _F32, tag="psA")
                psB = qpool.tile([128, 392], _F32, tag="psB")
                patch = ppool.tile([128, nslots * C], _F32,
                                   tag=f"patch{nslots}")
                # one indirect DMA per 128-pixel slot: HW multi-index-per-
                # partition offset tensors were observed to gather wrong
                # data (sim accepts them), so keep idx AP [128, 1].
                for q in range(nslots):
                    nc.gpsimd.indirect_dma_start(
                        out=patch[:, q * C:(q + 1) * C], out_offset=None,
                        in_=feat[:, :],
                        in_offset=bass.IndirectOffsetOnAxis(
                            ap=idx_sb[:, s0 + q:s0 + q + 1], axis=0))
                if kind == "TP":
                    w2d = wpool.tile([128, 2 * TT], _F32, tag="w2dp")
                    w2d_build(w2d[:], s0, 2)
                    nc.tensor.matmul(psA[:, 0:2 * TT], lhsT=patch[:, 0:128],
                                     rhs=w2d[:], start=True, stop=True)
                    nc.tensor.matmul(psB[:, 0:2 * TT], lhsT=patch[:, 128:256],
                                     rhs=w2d[:], start=True, stop=True)
                else:
                    for q in range(nslots):
                        w2d = wpool.tile([128, TT], _F32, tag="w2ds")
                        w2d_build(w2d[:], s0 + q, 1)
                        st = (q == 0)
                        sp = (q == nslots - 1)
                        nc.tensor.matmul(
                            psA[:, 0:TT],
                            lhsT=patch[:, q * C:q * C + 128],
                            rhs=w2d[:], start=st, stop=sp)
                        nc.tensor.matmul(
                            psB[:, 0:TT],
                            lhsT=patch[:, q * C + 128:(q + 1) * C],
                            rhs=w2d[:], start=st, stop=sp)
                # stage layout: per row: [c0_half (196) | c1_half (196)]
                stage = spool.tile([128, nrow * 2 * TT], _F32,
                                   tag=f"stage{nrow}")
                for r in range(nrow):
                    nc.vector.tensor_copy(
                        out=stage[:, r * 2 * TT:r * 2 * TT + TT],
                        in_=psA[:, r * TT:(r + 1) * TT])
                    nc.scalar.copy(
                        out=stage[:, r * 2 * TT + TT:(r + 1) * 2 * TT],
                        in_=psB[:, r * TT:(r + 1) * TT])
                nc.sync.dma_start(
                    out=out[r0:r0 + nrow]
                        .rearrange("n (h p) t -> p n h t", p=128),
                    in_=stage[:]
                        .rearrange("p (n h t) -> p n h t", h=2, t=TT))
    nc.finalize()
    return nc


_NC_CACHE = {}


def _get_program():
    key = "main"
    if key not in _NC_CACHE:
        _NC_CACHE[key] = build_bass_program()
    return _NC_CACHE[key]


# ---------------------------------------------------------------- entry
def _prepare(p2, p3, p4, p5, proposals):
    feats = [np.asarray(p, np.float32) for p in (p2, p3, p4, p5)]
    featcat = np.concatenate(
        [np.ascontiguousarray(f.transpose(0, 2, 3, 1)).reshape(-1, C)
         for f in feats], axis=0)
    plans, _lvl = _plan_proposals(proposals)
    placement, fallback = _assign_cores(plans)
    idxT, wyx = _build_slot_tensors(plans, placement)
    return featcat, plans, placement, fallback, idxT, wyx


def run(p2, p3, p4, p5, proposals, trace=False):
    featcat, plans, placement, fallback, idxT, wyx = _prepare(
        p2, p3, p4, p5, proposals)
    nc = _get_program()
    in_maps = [
        {"featcat": featcat, "idx": idxT[c], "wyx": wyx[c]}
        for c in range(NCORES)
    ]
    res = bass_utils.run_bass_kernel_spmd(
        nc, in_maps, core_ids=list(range(NCORES)), trace=trace)
    out = np.empty((NPROP, C, CS, CS), np.float32)
    done = np.zeros(NPROP, bool)
    for core in range(NCORES):
        core_out = res.results[core]["out"]  # [NROWS, 256, 196]
        for g in range(NG):
            occ = placement[core][g]
            if not occ:
                continue
            r0 = int(GROUP_ROW[g])
            for sub, pid in enumerate(occ):
                out[pid] = core_out[r0 + sub].reshape(C, CS, CS)
                done[pid] = True
    for pid in fallback:
        out[pid] = _host_crop(featcat, plans, pid)
        done[pid] = True
    assert done.all(), "some proposals unassigned"
    return out, res


def kernel(p2, p3, p4, p5, proposals):
    out, _res = run(p2, p3, p4, p5, proposals, trace=False)
    return out


# revision 9
# speedup vs baseline: 1.6541x; 1.1321x over previous
"""Trainium2 Bass kernel for nn_CropRoi (FPN ROI crop / roi-align style).

Contract: kernel(**inputs) takes the FULL inputs (p2..p5 feature pyramid,
proposals [1024, 7]) and returns the FULL output [1024, 256, 14, 14] f32.

Strategy
--------
Host (numpy, proposal-derived only — cheap):
  * Transpose each pyramid level to channels-last and concatenate into one
    flat pixel table featcat [174080, 256] f32 (one pixel = 1KB contiguous).
  * Route each proposal to its level, compute its source patch bounding box
    and separable bilinear weights wy[row, i], wx[col, j].
  * Flatten each patch to a pixel list (hp*wp pixels, mean ~45) and pack
    into 128-pixel slots. Small proposals are PAIRED two-per-slot (block
    structure handled by host-zeroed weight factors); big ones span 2-3
    slots, accumulated in PSUM.
  * Distribute proposals over 8 cores into a STATIC per-core group
    schedule: TP pair-groups (1 slot, 2 props), T1 solo (1 slot),
    T2 (2 slots), T3 (3 slots).

Device (one static SPMD Bass/Tile program on 8 cores):
  Per group: one indirect-DMA gather of its 128*k pixels (1KB descriptors,
  channels-last) into SBUF [128 pix, k*256 ch]; per slot build the 2D
  interp weight matrix W2D[pix, (g,i,j)] = wy[pix,(g,i)]*wx[pix,(g,j)]
  with one DVE broadcast-multiply; two fp32 matmuls patch.T @ W2D
  accumulate the crop(s) [256 ch, g*196] in PSUM; copy PSUM -> SBUF
  (DVE + ACT split) and DMA contiguous crops out.
"""

import os
import sys

for _p in ("/opt/trn_rl_repo",):
    if os.path.isdir(_p) and _p not in sys.path:
        sys.path.insert(0, _p)

import numpy as np

import concourse.bass as bass
import concourse.bacc as bacc
import concourse.mybir as mybir
from concourse.tile import TileContext
from concourse import bass_utils

# ---------------------------------------------------------------- constants
IMG = 1024
CS = 14  # crop size
TT = CS * CS  # 196
STRIDES = (4, 8, 16, 32)
BASE_SIZES = (8.0, 16.0, 32.0, 64.0)
B = 2
C = 256
NPROP = 1024
NCORES = 8
PPC = NPROP // NCORES  # proposals per core = 128

HWL = [IMG // s for s in STRIDES]  # [256, 128, 64, 32]
NPIXL = [B * h * h for h in HWL]
LEVEL_OFF = np.cumsum([0] + NPIXL)[:4]
TOTALPIX = int(sum(NPIXL))  # 174080

SLOT_PIX = 128

# static per-core group schedule: (kind, nslots, nrows)
N_TP, N_T1, N_T2, N_T3 = 59, 8, 11, 2
GROUPS = ([("TP", 1, 2)] * N_TP + [("T1", 1, 1)] * N_T1
          + [("T2", 2, 1)] * N_T2 + [("T3", 3, 1)] * N_T3)
NG = len(GROUPS)
NSLOT = sum(g[1] for g in GROUPS)       # 95
NROWS = sum(g[2] for g in GROUPS)       # 139
GROUP_SLOT = np.cumsum([0] + [g[1] for g in GROUPS])[:-1]
GROUP_ROW = np.cumsum([0] + [g[2] for g in GROUPS])[:-1]

_F32 = mybir.dt.float32
_I32 = mybir.dt.int32

# matmul-operand dtype: float32 = exact (2-pass PE), float16 = 1-pass PE
# (~7e-4 rel err), switchable for speed/precision tradeoff experiments.
MM_DTYPE = (mybir.dt.float16 if os.environ.get("CROP_MM_DTYPE", "f16") == "f16"
            else mybir.dt.float32)
MM_NP = np.float16 if MM_DTYPE == mybir.dt.float16 else np.float32


# ---------------------------------------------------------------- planner
def _plan_proposals(proposals):
    """Per-proposal gather indices + separable bilinear weight factors."""
    pr = np.asarray(proposals, dtype=np.float32)
    n = pr.shape[0]
    bi = pr[:, 0].astype(np.int32)
    x0, y0, x1, y1 = pr[:, 1], pr[:, 2], pr[:, 3], pr[:, 4]
    sizes = np.sqrt((x1 - x0) * (y1 - y0))
    base = np.asarray(BASE_SIZES, np.float32)
    lvl = np.argmin(np.abs(sizes[:, None] - base[None, :]), axis=1)
    grid = (np.arange(CS, dtype=np.float32) / np.float32(CS - 1))
    ar = np.arange(CS)

    plans = []
    for i in range(n):
        l = int(lvl[i])
        H = HWL[l]
        s = np.float32(1.0 / STRIDES[l])
        ys = y0[i] * s + (y1[i] - y0[i]) * s * grid
        xs = x0[i] * s + (x1[i] - x0[i]) * s * grid
        yf = np.floor(ys)
        xf = np.floor(xs)
        ly = ys - yf
        lx = xs - xf
        yi0 = np.clip(yf.astype(np.int64), 0, H - 1)
        yi1 = np.clip(yi0 + 1, 0, H - 1)
        xi0 = np.clip(xf.astype(np.int64), 0, H - 1)
        xi1 = np.clip(xi0 + 1, 0, H - 1)
        ylo = int(yi0.min())
        hp = int(yi1.max()) - ylo + 1
        xlo = int(xi0.min())
        wp = int(xi1.max()) - xlo + 1
        wyrow = np.zeros((hp, CS), np.float32)
        np.add.at(wyrow, (yi0 - ylo, ar), 1.0 - ly)
        np.add.at(wyrow, (yi1 - ylo, ar), ly)
        wxcol = np.zeros((wp, CS), np.float32)
        np.add.at(wxcol, (xi0 - xlo, ar), 1.0 - lx)
        np.add.at(wxcol, (xi1 - xlo, ar), lx)
        pbase = int(LEVEL_OFF[l]) + int(bi[i]) * H * H
        idx = (pbase + (ylo + np.arange(hp))[:, None] * H
               + (xlo + np.arange(wp))[None, :]).reshape(-1).astype(np.int32)
        wy2 = np.repeat(wyrow, wp, axis=0)  # [hp*wp, 14]
        wx2 = np.tile(wxcol, (hp, 1))       # [hp*wp, 14]
        plans.append((idx, wy2, wx2))
    return plans, lvl


def _assign_cores(plans):
    """Assign proposals to (core, group, sub) positions.

    Returns placement[core][g] = list of prop ids occupying that group
    (len 1 or 2 for TP), and the host-fallback list.
    """
    n = len(plans)
    pix = np.array([len(p[0]) for p in plans])
    ch = np.minimum((pix + SLOT_PIX - 1) // SLOT_PIX, 9)
    fallback = [i for i in range(n) if ch[i] > 3]

    core_ids = [[] for _ in range(NCORES)]
    for need in (3, 2, 1):
        ids = [i for i in range(n) if ch[i] == need and i not in fallback]
        # deal round-robin, smallest cores first for balance
        order = np.argsort([len(c) for c in core_ids], kind="stable")
        j = 0
        for i in ids:
            core_ids[order[j % NCORES]].append(i)
            j += 1

    placement = [[None] * NG for _ in range(NCORES)]
    kinds = [g[0] for g in GROUPS]
    tp_g = [i for i, k in enumerate(kinds) if k == "TP"]
    t1_g = [i for i, k in enumerate(kinds) if k == "T1"]
    t2_g = [i for i, k in enumerate(kinds) if k == "T2"]
    t3_g = [i for i, k in enumerate(kinds) if k == "T3"]

    for core in range(NCORES):
        ids = core_ids[core]
        free_tp = list(tp_g)
        free_t1 = list(t1_g)
        free_t2 = list(t2_g)
        free_t3 = list(t3_g)
        c3 = [i for i in ids if ch[i] == 3]
        c2 = [i for i in ids if ch[i] == 2]
        c1 = sorted((i for i in ids if ch[i] == 1), key=lambda i: pix[i])
        for i in c3:
            if free_t3:
                placement[core][free_t3.pop(0)] = [i]
            else:
                fallback.append(i)
        for i in c2:
            if free_t2:
                placement[core][free_t2.pop(0)] = [i]
            elif free_t3:
                placement[core][free_t3.pop(0)] = [i]
            else:
                fallback.append(i)
        # two-pointer pairing of the 1-chunk proposals
        lo, hi = 0, len(c1) - 1
        pairs, solos = [], []
        while lo < hi:
            if pix[c1[lo]] + pix[c1[hi]] <= SLOT_PIX:
                pairs.append((c1[lo], c1[hi]))
                lo += 1
                hi -= 1
            else:
                solos.append(c1[hi])
                hi -= 1
        if lo == hi:
            solos.append(c1[lo])
        for a, b in pairs:
            if free_tp:
                placement[core][free_tp.pop(0)] = [a, b]
            else:
                solos.extend((a, b))
        for i in solos:
            if free_t1:
                placement[core][free_t1.pop(0)] = [i]
            elif free_tp:
                placement[core][free_tp.pop(0)] = [i]
            elif free_t2:
                placement[core][free_t2.pop(0)] = [i]
            elif free_t3:
                placement[core][free_t3.pop(0)] = [i]
            else:
                fallback.append(i)
    return placement, fallback


def _build_slot_tensors(plans, placement):
    idxT = np.zeros((NCORES, 128, NSLOT), np.int32)
    wyx = np.zeros((NCORES, 128, NSLOT * 56), np.float32)
    for core in range(NCORES):
        for g in range(NG):
            occ = placement[core][g]
            if not occ:
                continue
            kind, nslots, _ = GROUPS[g]
            s0 = int(GROUP_SLOT[g])
            if kind == "TP":
                # sub-proposals share slot s0; rows stacked, weight col
                # block per sub (host-zeroed cross blocks)
                row = 0
                for sub, pid in enumerate(occ):
                    idx, wy2, wx2 = plans[pid]
                    npx = len(idx)
                    idxT[core, row:row + npx, s0] = idx
                    cw = s0 * 56 + sub * 14
                    wyx[core, row:row + npx, cw:cw + 14] = wy2
                    wyx[core, row:row + npx, cw + 28:cw + 42] = wx2
                    row += npx
            else:
                pid = occ[0]
                idx, wy2, wx2 = plans[pid]
                npx = len(idx)
                for q in range(nslots):
                    a = q * SLOT_PIX
                    if a >= npx:
                        break
                    b = min(npx, a + SLOT_PIX)
                    cnt = b - a
                    s = s0 + q
                    idxT[core, :cnt, s] = idx[a:b]
                    wyx[core, :cnt, s * 56:s * 56 + 14] = wy2[a:b]
                    wyx[core, :cnt, s * 56 + 28:s * 56 + 42] = wx2[a:b]
    return idxT, wyx


def _host_crop(featcat, plans, pid):
    """Numpy fallback for proposals that exceed slot capacity (rare)."""
    idx, wy2, wx2 = plans[pid]
    w2d = (wy2[:, :, None] * wx2[:, None, :]).reshape(len(idx), TT)
    out = featcat[idx].T @ w2d
    return out.reshape(C, CS, CS).astype(np.float32)


# ---------------------------------------------------------------- device
def build_bass_program(totalpix=TOTALPIX, groups=None):
    groups = GROUPS if groups is None else groups
    nslot = sum(g[1] for g in groups)
    nrows = sum(g[2] for g in groups)
    g_slot = np.cumsum([0] + [g[1] for g in groups])[:-1]
    g_row = np.cumsum([0] + [g[2] for g in groups])[:-1]

    nc = bacc.Bacc("TRN2", target_bir_lowering=False)
    feat = nc.dram_tensor("featcat", [totalpix, C], _F32, kind="ExternalInput")
    idx = nc.dram_tensor("idx", [128, nslot], _I32, kind="ExternalInput")
    wyx = nc.dram_tensor("wyx", [128, nslot * 56], MM_DTYPE,
                         kind="ExternalInput")
    out = nc.dram_tensor("out", [nrows, C, TT], _F32, kind="ExternalOutput")

    def w2d_build(w2d_ap, s, ngrp):
        # W2D[p, (g,i,j)] = wy[p, (g,i)] * wx[p, (g,j)]
        wy = wyx_sb[:, s * 56:s * 56 + 14 * ngrp]
        wx = wyx_sb[:, s * 56 + 28:s * 56 + 28 + 14 * ngrp]
        wyb = wy.rearrange("p (g i) -> p g i", g=ngrp) \
            .unsqueeze(3).to_broadcast([128, ngrp, CS, CS])
        wxb = wx.rearrange("p (g j) -> p g j", g=ngrp) \
            .unsqueeze(2).to_broadcast([128, ngrp, CS, CS])
        nc.vector.tensor_tensor(
            out=w2d_ap.rearrange("p (g i j) -> p g i j", g=ngrp, i=CS),
            in0=wyb, in1=wxb, op=mybir.AluOpType.mult)

    with TileContext(nc) as tc:
        with tc.tile_pool(name="const", bufs=1) as cpool, \
             tc.tile_pool(name="patch", bufs=8) as ppool, \
             tc.tile_pool(name="w2d", bufs=8) as wpool, \
             tc.tile_pool(name="stage", bufs=6) as spool, \
             tc.tile_pool(name="psum", bufs=3, space="PSUM") as qpool:
            idx_sb = cpool.tile([128, nslot], _I32)
            nc.sync.dma_start(out=idx_sb[:], in_=idx[:, :])
            wyx_sb = cpool.tile([128, nslot * 56], MM_DTYPE)
            nc.sync.dma_start(out=wyx_sb[:], in_=wyx[:, :])

            for g, (kind, nslots, nrow) in enumerate(groups):
                s0 = int(g_slot[g])
                r0 = int(g_row[g])
                psA = qpool.tile([128, 392], _F32, tag="psA")
                psB = qpool.tile([128, 392], _F32, tag="psB")
                patch = ppool.tile([128, nslots * C], MM_DTYPE,
                                   tag=f"patch{nslots}")
                # one indirect DMA per 128-pixel slot: HW multi-index-per-
                # partition offset tensors were observed to gather wrong
                # data (sim accepts them), so keep idx AP [128, 1].
                for q in range(nslots):
                    nc.gpsimd.indirect_dma_start(
                        out=patch[:, q * C:(q + 1) * C], out_offset=None,
                        in_=feat[:, :],
                        in_offset=bass.IndirectOffsetOnAxis(
                            ap=idx_sb[:, s0 + q:s0 + q + 1], axis=0))
                if kind == "TP":
                    w2d = wpool.tile([128, 2 * TT], MM_DTYPE, tag="w2dp")
                    w2d_build(w2d[:], s0, 2)
                    nc.tensor.matmul(psA[:, 0:2 * TT], lhsT=patch[:, 0:128],
                                     rhs=w2d[:], start=True, stop=True)
                    nc.tensor.matmul(psB[:, 0:2 * TT], lhsT=patch[:, 128:256],
                                     rhs=w2d[:], start=True, stop=True)
                else:
                    for q in range(nslots):
                        w2d = wpool.tile([128, TT], MM_DTYPE, tag="w2ds")
                        w2d_build(w2d[:], s0 + q, 1)
                        st = (q == 0)
                        sp = (q == nslots - 1)
                        nc.tensor.matmul(
                            psA[:, 0:TT],
                            lhsT=patch[:, q * C:q * C + 128],
                            rhs=w2d[:], start=st, stop=sp)
                        nc.tensor.matmul(
                            psB[:, 0:TT],
                            lhsT=patch[:, q * C + 128:(q + 1) * C],
                            rhs=w2d[:], start=st, stop=sp)
                # stage layout: per row: [c0_half (196) | c1_half (196)]
                stage = spool.tile([128, nrow * 2 * TT], _F32,
                                   tag=f"stage{nrow}")
                for r in range(nrow):
                    nc.vector.tensor_copy(
                        out=stage[:, r * 2 * TT:r * 2 * TT + TT],
                        in_=psA[:, r * TT:(r + 1) * TT])
                    nc.scalar.copy(
                        out=stage[:, r * 2 * TT + TT:(r + 1) * 2 * TT],
                        in_=psB[:, r * TT:(r + 1) * TT])
                nc.sync.dma_start(
                    out=out[r0:r0 + nrow]
                        .rearrange("n (h p) t -> p n h t", p=128),
                    in_=stage[:]
                        .rearrange("p (n h t) -> p n h t", h=2, t=TT))
    nc.finalize()
    return nc


_NC_CACHE = {}


def _get_program():
    key = ("main", str(MM_DTYPE))
    if key not in _NC_CACHE:
        _NC_CACHE[key] = build_bass_program()
    return _NC_CACHE[key]


# ---------------------------------------------------------------- entry
def _prepare(p2, p3, p4, p5, proposals):
    feats = [np.asarray(p, np.float32) for p in (p2, p3, p4, p5)]
    featcat = np.concatenate(
        [np.ascontiguousarray(f.transpose(0, 2, 3, 1)).reshape(-1, C)
         for f in feats], axis=0)
    plans, _lvl = _plan_proposals(proposals)
    placement, fallback = _assign_cores(plans)
    idxT, wyx = _build_slot_tensors(plans, placement)
    return featcat, plans, placement, fallback, idxT, wyx


def run(p2, p3, p4, p5, proposals, trace=False):
    featcat, plans, placement, fallback, idxT, wyx = _prepare(
        p2, p3, p4, p5, proposals)
    nc = _get_program()
    in_maps = [
        {"featcat": featcat, "idx": idxT[c], "wyx": wyx[c].astype(MM_NP)}
        for c in range(NCORES)
    ]
    res = bass_utils.run_bass_kernel_spmd(
        nc, in_maps, core_ids=list(range(NCORES)), trace=trace)
    out = np.empty((NPROP, C, CS, CS), np.float32)
    done = np.zeros(NPROP, bool)
    for core in range(NCORES):
        core_out = res.results[core]["out"]  # [NROWS, 256, 196]
        for g in range(NG):
            occ = placement[core][g]
            if not occ:
                continue
            r0 = int(GROUP_ROW[g])
            for sub, pid in enumerate(occ):
                out[pid] = core_out[r0 + sub].reshape(C, CS, CS)
                done[pid] = True
    for pid in fallback:
        out[pid] = _host_crop(featcat, plans, pid)
        done[pid] = True
    assert done.all(), "some proposals unassigned"
    return out, res


def kernel(p2, p3, p4, p5, proposals):
    out, _res = run(p2, p3, p4, p5, proposals, trace=False)
    return out
